# revision 17
# baseline (speedup 1.0000x reference)
"""GCN (2-layer) Trainium2 kernel, 8-core SPMD.

Strategy: partition destination nodes across 8 cores (contiguous shards of
12544). Both GCNConv layers are computed as aggregate-then-transform:
    out = dinv * ( (sum_{src in N(dst)} table[src]) @ W + sqrt(deg)*b )
with table = dinv-prescaled features (so the symmetric deg^-1/2 norm is exact).
Per 128-dst window the aggregation is a PE segment-matmul:
    aggT[feat, dst] += M[msgs, feat].T @ S[msgs, dst]
where M is a dma_gather of source rows (bf16, 256B each) and S is a one-hot
built on DVE via is_equal(iota, dst_rel). h (post-relu, bf16) is exchanged
between layers with a single AllGather. The same edge metadata (indices,
dst_rel, tile structure) is shared by both layers.
"""

import sys

sys.path.insert(0, "/opt/trn_rl_repo")

import numpy as np
import ml_dtypes

import concourse.bass as bass
import concourse.mybir as mybir
from concourse import bacc
from concourse.tile import TileContext

N = 100000
E = 600000
IN_D = 128
HID_D = 128
OUT_D = 64
NCORES = 8
BLK = 128               # dst window size
NW = 98                 # windows per shard
SH = BLK * NW           # 12544 nodes per shard
NPADN = NCORES * SH     # 100352 padded node count
CHUNK = NPADN // 4      # gather chunk rows (25088; must be < 32768 for int16)
NCHUNK = 4              # ceil(NPADN / CHUNK)
BATCH_W = 14            # windows per gather batch
NBATCH = NW // BATCH_W  # 7
GCAP = 32               # max tiles (x128 idxs) per dma_gather call (needs single_packet=False)

f32 = mybir.dt.float32
bf16 = mybir.dt.bfloat16
i16 = mybir.dt.int16
bf16np = ml_dtypes.bfloat16

_cache = {}


def _preprocess(edge_index):
    """Build per-core padded message streams + the shared static structure."""
    src_all = np.concatenate([edge_index[0], np.arange(N, dtype=np.int64)])
    dst_all = np.concatenate([edge_index[1], np.arange(N, dtype=np.int64)])
    deg = np.bincount(dst_all, minlength=N).astype(np.float32)  # >= 1 (self loop)
    dinv = 1.0 / np.sqrt(deg)
    sdeg = np.sqrt(deg)

    # message streams: edges only (self-loops become per-window identity tiles)
    src_e = edge_index[0]
    dst_e = edge_index[1]
    HS = SH // 2  # half-shard rows
    r_all = src_e // SH
    row_all = src_e % SH
    posmap_src = np.where(row_all < HS, r_all * HS + row_all,
                          NCORES * HS + r_all * HS + (row_all - HS))
    core = dst_e // SH
    w_all = (dst_e % SH) // BLK
    k_all = posmap_src // CHUNK
    grp_all = w_all * NCHUNK + k_all  # 392 groups per core

    # group processing order: for b in batches, for k in chunks, for w in batch
    order_groups = []
    for b in range(NBATCH):
        for k in range(NCHUNK):
            for w in range(b * BATCH_W, (b + 1) * BATCH_W):
                order_groups.append(w * NCHUNK + k)
    order_groups = np.asarray(order_groups)
    grank = np.empty(NW * NCHUNK, np.int64)
    grank[order_groups] = np.arange(NW * NCHUNK)

    counts = np.zeros((NCORES, NW * NCHUNK), np.int64)
    per_core = []
    for c in range(NCORES):
        m = core == c
        s, g = posmap_src[m], grp_all[m]
        d = dst_e[m]
        counts[c] = np.bincount(g, minlength=NW * NCHUNK)
        per_core.append((s, d, g))

    T_wk = np.maximum((counts.max(axis=0) + BLK - 1) // BLK, 1)  # tiles per group
    # offsets in tile stream, in group processing order
    tiles_ord = T_wk[order_groups]
    tile_off_ord = np.concatenate([[0], np.cumsum(tiles_ord)])
    T_total = int(tile_off_ord[-1])
    tile_off = np.empty(NW * NCHUNK, np.int64)  # per group, in group-id space
    tile_off[order_groups] = tile_off_ord[:-1]
    L = T_total * BLK  # total message slots

    idx_arrs = np.zeros((NCORES, 128, L // 16), np.int16)
    dstrel_arrs = np.full((NCORES, 128, T_total), 300.0, np.float32)
    for c in range(NCORES):
        s, d, g = per_core[c]
        o = np.argsort(g, kind="stable")
        s, d, g = s[o], d[o], g[o]
        gstart = np.concatenate([[0], np.cumsum(np.bincount(g, minlength=NW * NCHUNK))])
        within = np.arange(len(g)) - gstart[g]
        pos = tile_off[g] * BLK + within
        rel = (s % CHUNK).astype(np.int16)
        idxw = np.zeros(L, np.int16)
        idxw[pos] = rel
        idxw = idxw.reshape(L // 16, 16).T  # [16, L/16]
        idx_arrs[c] = np.tile(idxw, (8, 1))
        drel = np.full(L, 300.0, np.float32)
        drel[pos] = (d % BLK).astype(np.float32)
        dstrel_arrs[c] = drel.reshape(T_total, 128).T  # [128, T]

    # static structure for codegen
    calls = []  # (b, k, ntiles, tile_start)
    for b in range(NBATCH):
        for k in range(NCHUNK):
            gs = [(b * BATCH_W + i) * NCHUNK + k for i in range(BATCH_W)]
            nt = int(T_wk[gs].sum())
            calls.append((b, k, nt, int(tile_off[gs[0]])))
    # per window: list of (chunk, column offset within that (b,k) call buffer)
    win_tiles = []
    for w in range(NW):
        b = w // BATCH_W
        lst = []
        for k in range(NCHUNK):
            gs0 = (b * BATCH_W) * NCHUNK + k
            coloff = int(tile_off[w * NCHUNK + k] - tile_off[gs0])
            for j in range(int(T_wk[w * NCHUNK + k])):
                lst.append((k, coloff + j))
        win_tiles.append(lst)

    g_ids = np.arange(NPADN)
    r_g = g_ids // SH
    row_g = g_ids % SH
    pos_g = np.where(row_g < HS, r_g * HS + row_g,
                     NCORES * HS + r_g * HS + (row_g - HS))
    return dict(
        pos_g=pos_g,
        deg=deg, dinv=dinv, sdeg=sdeg, T_total=T_total, L=L,
        idx_arrs=idx_arrs, dstrel_arrs=dstrel_arrs, calls=calls,
        win_tiles=win_tiles,
    )


def _build_nc(meta):
    T_total = meta["T_total"]
    L = meta["L"]
    calls = meta["calls"]
    win_tiles = meta["win_tiles"]

    nc = bacc.Bacc(None, target_bir_lowering=False, dynamic_dma_scratch_size=65536)

    xs_d = nc.dram_tensor("xs", [NPADN, IN_D], bf16, kind="ExternalInput")
    xso_d = nc.dram_tensor("xso", [SH, IN_D], bf16, kind="ExternalInput")
    pidx_d = nc.dram_tensor("pidx", [128, 1], f32, kind="ExternalInput")
    idx_d = nc.dram_tensor("idx", [128, L // 16], i16, kind="ExternalInput")
    dstrel_d = nc.dram_tensor("dstrel", [128, T_total], f32, kind="ExternalInput")
    iota_d = nc.dram_tensor("iota", [128, BLK], mybir.dt.float16, kind="ExternalInput")
    w1_d = nc.dram_tensor("w1", [IN_D, HID_D], bf16, kind="ExternalInput")
    w2_d = nc.dram_tensor("w2", [HID_D, OUT_D], bf16, kind="ExternalInput")
    b1_d = nc.dram_tensor("b1", [1, HID_D], f32, kind="ExternalInput")
    b2_d = nc.dram_tensor("b2", [1, OUT_D], f32, kind="ExternalInput")
    dinv_d = nc.dram_tensor("dinv", [128, NW], f32, kind="ExternalInput")
    dinv2_d = nc.dram_tensor("dinv2", [128, NW], f32, kind="ExternalInput")
    sdeg_d = nc.dram_tensor("sdeg", [1, SH], f32, kind="ExternalInput")
    out_d = nc.dram_tensor("out", [SH, OUT_D], f32, kind="ExternalOutput")

    HS = SH // 2
    h_send_a = nc.dram_tensor("h_send_a", [HS, HID_D], bf16)
    h_send_b = nc.dram_tensor("h_send_b", [HS, HID_D], bf16)
    h_full_a = nc.dram_tensor("h_full_a", [NCORES * HS, HID_D], bf16,
                              addr_space="Shared")
    h_full_b = nc.dram_tensor("h_full_b", [NCORES * HS, HID_D], bf16,
                              addr_space="Shared")

    with TileContext(nc) as tc:
        with (
            tc.tile_pool(name="const", bufs=1) as constp,
            tc.tile_pool(name="gath", bufs=2) as gathp,
            tc.tile_pool(name="sbuild", bufs=4) as sp,
            tc.tile_pool(name="agg", bufs=3) as aggp,
            tc.tile_pool(name="outp", bufs=3) as outp,
            tc.tile_pool(name="psum_seg", bufs=3, space="PSUM") as psegp,
            tc.tile_pool(name="psum_h", bufs=2, space="PSUM") as phk,
        ):
            # persistent tiles
            idx_t = constp.tile([128, L // 16], i16, tag="idx")
            dstrel_t = constp.tile([128, T_total], f32, tag="dstrel")
            iota_t = constp.tile([128, BLK], mybir.dt.float16, tag="iota")
            w1_t = constp.tile([IN_D, HID_D], bf16, tag="w1")
            w2_t = constp.tile([HID_D, OUT_D], bf16, tag="w2")
            b1_t = constp.tile([1, HID_D], f32, tag="b1")
            b2_t = constp.tile([1, OUT_D], f32, tag="b2")
            dinv_t = constp.tile([128, NW], f32, tag="dinv")
            dinv2_t = constp.tile([128, NW], f32, tag="dinv2")
            sdeg_t = constp.tile([1, SH], f32, tag="sdeg")
            pidx_t = constp.tile([128, 1], f32, tag="pidx")
            s_id = constp.tile([128, BLK], bf16, tag="s_id")
            nc.sync.dma_start(out=idx_t[:], in_=idx_d[:])
            nc.sync.dma_start(out=dstrel_t[:], in_=dstrel_d[:])
            nc.sync.dma_start(out=iota_t[:], in_=iota_d[:])
            nc.sync.dma_start(out=w1_t[:], in_=w1_d[:])
            nc.sync.dma_start(out=w2_t[:], in_=w2_d[:])
            nc.sync.dma_start(out=b1_t[:], in_=b1_d[:])
            nc.sync.dma_start(out=b2_t[:], in_=b2_d[:])
            nc.sync.dma_start(out=dinv_t[:], in_=dinv_d[:])
            nc.sync.dma_start(out=dinv2_t[:], in_=dinv2_d[:])
            nc.sync.dma_start(out=sdeg_t[:], in_=sdeg_d[:])
            nc.sync.dma_start(out=pidx_t[:], in_=pidx_d[:])
            nc.vector.tensor_scalar(
                s_id[:], iota_t[:], pidx_t[:, 0:1], None,
                mybir.AluOpType.is_equal,
            )

            def layer(tables, own_ap, w_t, b_t, od, out_dram, relu, out_dtype, scale_t, split_ab=False):
                for b in range(NBATCH):
                    bufs = {}
                    for (bb, k, nt, tstart) in calls:
                        if bb != b:
                            continue
                        g = gathp.tile([128, nt, IN_D], bf16, tag=f"g{k}")
                        for c0 in range(0, nt, GCAP):
                            ct = min(GCAP, nt - c0)
                            nidx = ct * BLK
                            nc.gpsimd.dma_gather(
                                g[:, c0:c0 + ct, :],
                                tables[k],
                                idx_t[:, (tstart + c0) * 8:
                                      (tstart + c0) * 8 + nidx // 16],
                                num_idxs=nidx, num_idxs_reg=nidx, elem_size=IN_D,
                                single_packet=False,
                            )
                        bufs[k] = g
                    for w in range(b * BATCH_W, (b + 1) * BATCH_W):
                        tl = win_tiles[w]

                        def seg_group(tl_part, tag, with_self):
                            pseg = psegp.tile([128, BLK], f32, tag=tag)
                            first = True
                            if with_self:
                                own = aggp.tile([128, IN_D], bf16, tag="own")
                                eng2 = nc.sync if (w % 2 == 1) else nc.scalar
                                if isinstance(own_ap, tuple):
                                    half = w // (NW // 2)
                                    wr = w % (NW // 2)
                                    eng2.dma_start(
                                        out=own[:],
                                        in_=own_ap[half][wr * BLK:(wr + 1) * BLK, :])
                                else:
                                    eng2.dma_start(
                                        out=own[:],
                                        in_=own_ap[w * BLK:(w + 1) * BLK, :])
                                nc.tensor.matmul(pseg[:], own[:], s_id[:],
                                                 start=True, stop=False)
                                first = False
                            for j, (k, col) in enumerate(tl_part):
                                s = sp.tile([128, BLK], bf16, tag="s")
                                tcol = None
                                for (bb, kk, nt2, ts2) in calls:
                                    if bb == b and kk == k:
                                        tcol = ts2 + col
                                        break
                                nc.vector.tensor_scalar(
                                    s[:], iota_t[:], dstrel_t[:, tcol:tcol + 1],
                                    None, mybir.AluOpType.is_equal,
                                )
                                nc.tensor.matmul(
                                    pseg[:], bufs[k][:, col, :], s[:],
                                    start=first,
                                    stop=(j == len(tl_part) - 1),
                                )
                                first = False
                            return pseg

                        ph = phk.tile([128, od], f32, tag="ph")
                        if split_ab:
                            tl_a = [t for t in tl if t[0] < 2]
                            tl_b = [t for t in tl if t[0] >= 2]
                            pseg_a = seg_group(tl_a, "psegA", True)
                            aggA = aggp.tile([128, BLK], bf16, tag="aggA")
                            nc.scalar.activation(
                                aggA[:], pseg_a[:], mybir.ActivationFunctionType.Copy)
                            nc.tensor.matmul(ph[:], aggA[:], w_t[:],
                                             start=True, stop=False)
                            pseg_b = seg_group(tl_b, "psegB", False)
                            aggB = aggp.tile([128, BLK], bf16, tag="aggB")
                            nc.scalar.activation(
                                aggB[:], pseg_b[:], mybir.ActivationFunctionType.Copy)
                            nc.tensor.matmul(ph[:], aggB[:], w_t[:],
                                             start=False, stop=False)
                        else:
                            pseg = seg_group(tl, "psegA", True)
                            aggT = aggp.tile([128, BLK], bf16, tag="aggA")
                            nc.scalar.activation(
                                aggT[:], pseg[:], mybir.ActivationFunctionType.Copy)
                            nc.tensor.matmul(ph[:], aggT[:], w_t[:],
                                             start=True, stop=False)
                        nc.tensor.matmul(
                            ph[:], sdeg_t[0:1, w * BLK:(w + 1) * BLK], b_t[:],
                            start=False, stop=True,
                        )
                        o = outp.tile([128, od], out_dtype, tag="o")
                        nc.scalar.activation(
                            o[:], ph[:],
                            mybir.ActivationFunctionType.Relu if relu
                            else mybir.ActivationFunctionType.Copy,
                            scale=scale_t[:, w:w + 1],
                        )
                        eng = nc.sync if (w % 2 == 0) else nc.scalar
                        if isinstance(out_dram, tuple):
                            half = w // (NW // 2)
                            wr = w % (NW // 2)
                            eng.dma_start(
                                out=out_dram[half][wr * BLK:(wr + 1) * BLK, :],
                                in_=o[:])
                        else:
                            eng.dma_start(
                                out=out_dram[w * BLK:(w + 1) * BLK, :], in_=o[:],
                            )

            def layer2(tables, own_ap, w_t, b_t, od, out_dram, scale_t):
                aggA = {}

                def sbuild(b, k, col):
                    s = sp.tile([128, BLK], bf16, tag="s")
                    tcol = None
                    for (bb, kk, nt2, ts2) in calls:
                        if bb == b and kk == k:
                            tcol = ts2 + col
                            break
                    nc.vector.tensor_scalar(
                        s[:], iota_t[:], dstrel_t[:, tcol:tcol + 1],
                        None, mybir.AluOpType.is_equal,
                    )
                    return s

                def gather_calls(b, ks):
                    bufs = {}
                    for (bb, k, nt, tstart) in calls:
                        if bb != b or k not in ks:
                            continue
                        g = gathp.tile([128, nt, IN_D], bf16, tag=f"g{k}")
                        for c0 in range(0, nt, GCAP):
                            ct = min(GCAP, nt - c0)
                            nidx = ct * BLK
                            nc.gpsimd.dma_gather(
                                g[:, c0:c0 + ct, :], tables[k],
                                idx_t[:, (tstart + c0) * 8:
                                      (tstart + c0) * 8 + nidx // 16],
                                num_idxs=nidx, num_idxs_reg=nidx,
                                elem_size=IN_D, single_packet=False,
                            )
                        bufs[k] = g
                    return bufs

                # pass A: chunks 0/1 + self tiles -> aggA (persistent SBUF)
                for b in range(NBATCH):
                    bufs = gather_calls(b, (0, 1))
                    for w in range(b * BATCH_W, (b + 1) * BATCH_W):
                        tl_a = [t for t in win_tiles[w] if t[0] < 2]
                        pseg = psegp.tile([128, BLK], f32, tag="psegA")
                        own = aggp.tile([128, IN_D], bf16, tag="own")
                        eng2 = nc.sync if (w % 2 == 1) else nc.scalar
                        half = w // (NW // 2)
                        wr = w % (NW // 2)
                        eng2.dma_start(
                            out=own[:],
                            in_=own_ap[half][wr * BLK:(wr + 1) * BLK, :])
                        nc.tensor.matmul(pseg[:], own[:], s_id[:],
                                         start=True, stop=False)
                        for j, (k, col) in enumerate(tl_a):
                            s = sbuild(b, k, col)
                            nc.tensor.matmul(
                                pseg[:], bufs[k][:, col, :], s[:],
                                start=False, stop=(j == len(tl_a) - 1),
                            )
                        ag = constp.tile([128, BLK], bf16, tag=f"aggA{w}")
                        nc.scalar.activation(
                            ag[:], pseg[:], mybir.ActivationFunctionType.Copy)
                        aggA[w] = ag
                # pass B: chunks 2/3, combine, transform, write out
                for b in range(NBATCH):
                    bufs = gather_calls(b, (2, 3))
                    for w in range(b * BATCH_W, (b + 1) * BATCH_W):
                        tl_b = [t for t in win_tiles[w] if t[0] >= 2]
                        pseg = psegp.tile([128, BLK], f32, tag="psegB")
                        for j, (k, col) in enumerate(tl_b):
                            s = sbuild(b, k, col)
                            nc.tensor.matmul(
                                pseg[:], bufs[k][:, col, :], s[:],
                                start=(j == 0), stop=(j == len(tl_b) - 1),
                            )
                        aggB = aggp.tile([128, BLK], bf16, tag="aggB")
                        nc.scalar.activation(
                            aggB[:], pseg[:], mybir.ActivationFunctionType.Copy)
                        ph = phk.tile([128, od], f32, tag="ph")
                        nc.tensor.matmul(ph[:], aggA[w][:], w_t[:],
                                         start=True, stop=False)
                        nc.tensor.matmul(ph[:], aggB[:], w_t[:],
                                         start=False, stop=False)
                        nc.tensor.matmul(
                            ph[:], sdeg_t[0:1, w * BLK:(w + 1) * BLK], b_t[:],
                            start=False, stop=True,
                        )
                        o = outp.tile([128, od], f32, tag="o")
                        nc.scalar.activation(
                            o[:], ph[:], mybir.ActivationFunctionType.Copy,
                            scale=scale_t[:, w:w + 1],
                        )
                        eng = nc.sync if (w % 2 == 0) else nc.scalar
                        eng.dma_start(
                            out=out_dram[w * BLK:(w + 1) * BLK, :], in_=o[:],
                        )

            xs_tables = [xs_d[k * CHUNK:(k + 1) * CHUNK, :] for k in range(NCHUNK)]
            layer(xs_tables, xso_d, w1_t, b1_t, HID_D, (h_send_a, h_send_b),
                  True, bf16, dinv2_t)
            if NCORES == 1:
                nc.sync.dma_start(out=h_full_a[:], in_=h_send_a[:])
                nc.sync.dma_start(out=h_full_b[:], in_=h_send_b[:])
            else:
                nc.gpsimd.collective_compute(
                    "AllGather", mybir.AluOpType.bypass,
                    replica_groups=[list(range(NCORES))],
                    ins=[h_send_a[:]], outs=[h_full_a[:]],
                )
                nc.gpsimd.collective_compute(
                    "AllGather", mybir.AluOpType.bypass,
                    replica_groups=[list(range(NCORES))],
                    ins=[h_send_b[:]], outs=[h_full_b[:]],
                )
            h_tables = [
                h_full_a[0:CHUNK, :], h_full_a[CHUNK:2 * CHUNK, :],
                h_full_b[0:CHUNK, :], h_full_b[CHUNK:2 * CHUNK, :],
            ]
            layer2(h_tables, (h_send_a, h_send_b), w2_t, b2_t, OUT_D, out_d,
                   dinv_t)

    nc.compile()
    return nc


def _get_runner(edge_index_bytes, edge_index):
    key = hash(edge_index_bytes)
    if key in _cache:
        return _cache[key]
    meta = _preprocess(edge_index.astype(np.int64))
    nc = _build_nc(meta)
    runner = _Runner(nc)
    _cache[key] = (meta, nc, runner)
    return _cache[key]


def _in_maps(meta, x, W1, b1, W2, b2):
    dinv = meta["dinv"]
    xs = (x * dinv[:, None]).astype(bf16np)
    xs = np.concatenate([xs, np.zeros((NPADN - N, IN_D), bf16np)], axis=0)
    xs_own_full = xs
    xs_r = np.empty_like(xs)
    xs_r[meta["pos_g"]] = xs
    xs = xs_r
    iota = np.broadcast_to(np.arange(BLK, dtype=np.float16), (128, BLK)).copy()
    dinv_p = np.concatenate([dinv, np.zeros(NPADN - N, np.float32)])
    sdeg_p = np.concatenate([meta["sdeg"], np.zeros(NPADN - N, np.float32)])
    maps = []
    for c in range(NCORES):
        dv = dinv_p[c * SH:(c + 1) * SH].reshape(NW, 128).T.copy()
        dv2 = (dv * dv).copy()
        sd = sdeg_p[c * SH:(c + 1) * SH].reshape(1, SH).copy()
        maps.append({
            "xs": xs,
            "xso": xs_own_full[c * SH:(c + 1) * SH],
            "pidx": np.arange(128, dtype=np.float32).reshape(128, 1),
            "idx": meta["idx_arrs"][c],
            "dstrel": meta["dstrel_arrs"][c],
            "iota": iota,
            "w1": W1.astype(bf16np),
            "w2": W2.astype(bf16np),
            "b1": b1.reshape(1, HID_D).astype(np.float32),
            "b2": b2.reshape(1, OUT_D).astype(np.float32),
            "dinv": dv,
            "dinv2": dv2,
            "sdeg": sd,
        })
    return maps


class _Runner:
    """Compile-once PJRT executor for a fixed Bass module (8-core SPMD)."""

    def __init__(self, nc):
        import jax
        from jax.sharding import Mesh, PartitionSpec
        from jax.experimental.shard_map import shard_map
        from concourse import bass2jax

        bass2jax.install_neuronx_cc_hook()
        self.nc = nc
        in_names, out_names, out_avals, zero_shapes = [], [], [], []
        pname = nc.partition_id_tensor.name if nc.partition_id_tensor else None
        for alloc in nc.m.functions[0].allocations:
            if not isinstance(alloc, mybir.MemoryLocationSet):
                continue
            name = alloc.memorylocations[0].name
            if alloc.kind == "ExternalInput":
                if name != pname:
                    in_names.append(name)
            elif alloc.kind == "ExternalOutput":
                out_names.append(name)
                shape = tuple(alloc.tensor_shape)
                dtype = mybir.dt.np(alloc.dtype)
                out_avals.append(jax.core.ShapedArray(shape, dtype))
                zero_shapes.append((shape, dtype))
        self.in_names, self.out_names = in_names, out_names
        self.zero_shapes = zero_shapes
        n_params, n_outs = len(in_names), len(out_names)
        all_names = in_names + out_names + ([pname] if pname else [])

        def _body(*args):
            operands = list(args)
            if pname is not None:
                operands.append(bass2jax.partition_id_tensor())
            outs = bass2jax._bass_exec_p.bind(
                *operands,
                out_avals=tuple(out_avals),
                in_names=tuple(all_names),
                out_names=tuple(out_names),
                lowering_input_output_aliases=(),
                sim_require_finite=True,
                sim_require_nnan=True,
                nc=nc,
            )
            return tuple(outs)

        devices = jax.devices()[:NCORES]
        mesh = Mesh(np.asarray(devices), ("core",))
        self.mesh = mesh
        in_specs = (PartitionSpec("core"),) * (n_params + n_outs)
        out_specs = (PartitionSpec("core"),) * n_outs
        self.fn = jax.jit(
            shard_map(_body, mesh=mesh, in_specs=in_specs, out_specs=out_specs,
                      check_rep=False),
            donate_argnums=tuple(range(n_params, n_params + n_outs)),
            keep_unused=True,
        )
        self.out_avals = out_avals

    def prep(self, in_maps):
        return [
            np.concatenate([np.asarray(in_maps[c][n]) for c in range(NCORES)],
                           axis=0)
            for n in self.in_names
        ]

    def zeros(self):
        return [np.zeros((NCORES * s[0], *s[1:]), d) for s, d in self.zero_shapes]

    def run_raw(self, concat_in, concat_zeros):
        import jax
        out_arrs = self.fn(*concat_in, *concat_zeros)
        jax.block_until_ready(out_arrs)
        return out_arrs

    def __call__(self, concat_in, concat_zeros):
        out_arrs = self.run_raw(concat_in, concat_zeros)
        return {
            n: np.asarray(out_arrs[i]).reshape(
                NCORES, *self.out_avals[i].shape)
            for i, n in enumerate(self.out_names)
        }


def kernel(x, edge_index, W1, b1, W2, b2):
    x = np.asarray(x, np.float32)
    edge_index = np.asarray(edge_index)
    meta, nc, runner = _get_runner(edge_index.tobytes(), edge_index)
    maps = _in_maps(meta, x, np.asarray(W1), np.asarray(b1), np.asarray(W2),
                    np.asarray(b2))
    res = runner(runner.prep(maps), runner.zeros())
    out = res["out"].reshape(NCORES * SH, OUT_D)
    return out[:N].astype(np.float32)
